# revision 2
# baseline (speedup 1.0000x reference)
"""Trainium2 Bass kernel v3 for nn_AutoCorrelation.

vs v2:
 - W=64 band (SPAN=256, 64-aligned spans): halves attention matmul columns
   and exp/mul area. Banding error ~4e-4, budget 2e-2.
 - Per-head batched scoring: all 16 score->exp->mul chains for a head are
   emitted before its PV matmuls, so PV is a dependency-free MM stream and the
   next head's scores fill remaining PE gaps (keeps HAM warm).
 - V projections and per-head-pair K projections moved into the attention
   phase as independent warm PE filler; xk is re-read per head pair (DMA has
   headroom).
 - Denominator dup-rows trick retained: V block is [V(64) | ones(64)] so the
   PV matmul replicates softmax sums into psum rows 64-127; ACT cross-base
   copy 64->0, DVE reciprocal + multiply, all base-aligned.
"""

import math
from contextlib import ExitStack

import numpy as np
import ml_dtypes

BF16 = ml_dtypes.bfloat16

N_CORES = 8


class Cfg:
    def __init__(self, L=2048, C=1024, NHL=8, DK=64, W=64):
        self.L, self.C, self.NHL, self.DK, self.W = L, C, NHL, DK, W
        self.DL = NHL * DK               # local head dims
        self.SPAN = 128 + 2 * W          # k-chunk q-span (64-aligned)
        self.KC = L // 128               # k chunks
        self.NB64 = L // 64              # 64-wide q blocks
        self.NQT = L // 512              # q tiles (512)
        self.CC = C // 128               # contraction chunks
        self.LT = L // 512               # l tiles
        self.HP = NHL // 2               # head pairs
        self.VW = NHL * 128              # V block width (V | 64x ones)
        self.EBW = self.SPAN + 512       # EB master width
        assert self.SPAN % 64 == 0 and self.SPAN <= L

    def qs_of(self, kc):
        return min(max(128 * kc - self.W, 0), self.L - self.SPAN)

    def covers64(self, qb):
        return [kc for kc in range(self.KC)
                if self.qs_of(kc) <= 64 * qb
                and self.qs_of(kc) + self.SPAN >= 64 * (qb + 1)]


FULL = Cfg(W=64)


def plan_pv(cfg):
    """64-granular PV matmul descriptors merged into per-(kc, qt) runs."""
    first_touch = {qb: min(cfg.covers64(qb)) for qb in range(cfg.NB64)}
    pv_mms = []          # (kc, qt_i, qoff, ncols, etb_off)
    qt_order = {qt: [] for qt in range(cfg.NQT)}
    for kc in range(cfg.KC):
        qs = cfg.qs_of(kc)
        qbs = [qs // 64 + j for j in range(cfg.SPAN // 64)]
        run = [qbs[0]]
        for qb in qbs[1:] + [None]:
            if (qb is not None and qb // 8 == run[0] // 8
                    and (first_touch[qb] == kc) == (first_touch[run[0]] == kc)):
                run.append(qb)
            else:
                qt_i = run[0] // 8
                qt_order[qt_i].append(len(pv_mms))
                pv_mms.append((kc, qt_i, (run[0] % 8) * 64, len(run) * 64,
                               (run[0] - qs // 64) * 64))
                run = [qb] if qb is not None else []
    qt_first = {qt: ids[0] for qt, ids in qt_order.items()}
    qt_last = {qt: ids[-1] for qt, ids in qt_order.items()}
    qt_done_at = {qt: pv_mms[ids[-1]][0] for qt, ids in qt_order.items()}
    return pv_mms, qt_first, qt_last, qt_done_at


def build_program(cfg=FULL, debug=False):
    import concourse.bass as bass
    import concourse.tile as tile
    from concourse import bacc, mybir

    f32 = mybir.dt.float32
    bf16 = mybir.dt.bfloat16
    AF = mybir.ActivationFunctionType

    L, C, NHL, DL = cfg.L, cfg.C, cfg.NHL, cfg.DL
    SPAN, KC, NQT, CC, LT, HP, VW = (cfg.SPAN, cfg.KC, cfg.NQT, cfg.CC,
                                     cfg.LT, cfg.HP, cfg.VW)

    nc = bacc.Bacc("TRN2", target_bir_lowering=False, debug=debug,
                   num_devices=N_CORES)

    xq = nc.dram_tensor("xq", [C, L], bf16, kind="ExternalInput").ap()
    xk = nc.dram_tensor("xk", [C, L], bf16, kind="ExternalInput").ap()
    xv = nc.dram_tensor("xv", [C, L], bf16, kind="ExternalInput").ap()
    wq = nc.dram_tensor("wq", [C, DL], bf16, kind="ExternalInput").ap()
    wk = nc.dram_tensor("wk", [C, DL], bf16, kind="ExternalInput").ap()
    wv = nc.dram_tensor("wv", [C, DL], bf16, kind="ExternalInput").ap()
    wo = nc.dram_tensor("wo", [DL, C], bf16, kind="ExternalInput").ap()
    bqd = nc.dram_tensor("bq", [128, HP], f32, kind="ExternalInput").ap()
    ebd = nc.dram_tensor("eb", [128, cfg.EBW], bf16, kind="ExternalInput").ap()
    out = nc.dram_tensor("out", [L, C], bf16, kind="ExternalOutput").ap()

    pv_mms, qt_first, qt_last, qt_done_at = plan_pv(cfg)
    mms_at = {kc: [i for i, m in enumerate(pv_mms) if m[0] == kc]
              for kc in range(KC)}

    def nsplit(total, cap=512):
        o, r = [], 0
        while r < total:
            n = min(cap, total - r)
            o.append((r, n))
            r += n
        return o

    with tile.TileContext(nc) as tc, ExitStack() as ctx:
        const = ctx.enter_context(tc.tile_pool(name="const", bufs=1))
        big = ctx.enter_context(tc.tile_pool(name="big", bufs=1))
        xs = ctx.enter_context(tc.tile_pool(name="xs", bufs=3))
        ets = ctx.enter_context(tc.tile_pool(name="ets", bufs=4))
        etbp = ctx.enter_context(tc.tile_pool(name="etbp", bufs=2))
        nrm = ctx.enter_context(tc.tile_pool(name="nrm", bufs=2))
        ostage = ctx.enter_context(tc.tile_pool(name="ostage", bufs=3))
        pp = ctx.enter_context(tc.tile_pool(name="pp", bufs=3, space="PSUM"))
        psc = ctx.enter_context(tc.tile_pool(name="psc", bufs=2, space="PSUM"))
        ppo = ctx.enter_context(tc.tile_pool(name="ppo", bufs=3, space="PSUM"))

        wq_sb = const.tile([128, CC * DL], bf16)
        wk_sb = const.tile([128, CC * DL], bf16)
        wv_sb = const.tile([128, CC * DL], bf16)
        wo_sb = const.tile([128, HP * C], bf16)
        eb_sb = const.tile([128, cfg.EBW], bf16)
        bq_sb = const.tile([128, HP], f32)
        qt_sb = [big.tile([128, L], bf16, name=f"qt{hp}") for hp in range(HP)]
        kt_sb = [big.tile([128, L], bf16, name=f"kt{hp}") for hp in range(HP)]
        vb_sb = big.tile([128, KC * VW], bf16)
        ots_sb = [big.tile([128, L], bf16, name=f"ots{hp}") for hp in range(HP)]

        nc.sync.dma_start(bq_sb[:], bqd[:])
        vb4 = vb_sb.rearrange("p (k h w) -> p k h w", h=NHL, w=128)
        nc.vector.memset(vb4[:, :, :, 64:128], 1.0)

        # ================= Phase A: Q projections =================
        for lt in range(LT):
            xq_sb = xs.tile([128, CC * 512], bf16, tag="xq", bufs=2, name=f"xq{lt}")
            for c in range(CC):
                if lt == 0:
                    nc.sync.dma_start(wq_sb[:, c * DL:(c + 1) * DL],
                                      wq[c * 128:(c + 1) * 128, :])
                nc.sync.dma_start(
                    xq_sb[:, c * 512:(c + 1) * 512],
                    xq[c * 128:(c + 1) * 128, lt * 512:(lt + 1) * 512])
            for hp in range(HP):
                ps = pp.tile([128, 512], f32, tag="psp", name=f"psp_q{lt}_{hp}")
                for c in range(CC):
                    nc.tensor.matmul(
                        ps[:],
                        lhsT=wq_sb[:, c * DL + hp * 128: c * DL + hp * 128 + 128],
                        rhs=xq_sb[:, c * 512:(c + 1) * 512],
                        start=(c == 0), stop=(c == CC - 1))
                nc.scalar.activation(qt_sb[hp][:, lt * 512:(lt + 1) * 512],
                                     ps[:], AF.Identity,
                                     bias=bq_sb[:, hp:hp + 1], scale=1.0)
        nc.sync.dma_start(eb_sb[:], ebd[:])
        for c in range(CC):
            nc.sync.dma_start(wv_sb[:, c * DL:(c + 1) * DL],
                              wv[c * 128:(c + 1) * 128, :])
        for c in range(CC):
            nc.sync.dma_start(wk_sb[:, c * DL:(c + 1) * DL],
                              wk[c * 128:(c + 1) * 128, :])

        # ================= Phase B prologue: V projections =================
        for lt in range(LT):
            xv_sb = xs.tile([128, CC * 512], bf16, tag="xv", bufs=2, name=f"xv{lt}")
            for c in range(CC):
                nc.sync.dma_start(
                    xv_sb[:, c * 512:(c + 1) * 512],
                    xv[c * 128:(c + 1) * 128, lt * 512:(lt + 1) * 512])
            for sub in range(4):
                kcg = lt * 4 + sub
                ps = pp.tile([128, DL], f32, tag="psp", name=f"psp_v{kcg}")
                for c in range(CC):
                    lhsT = xv_sb[:, c * 512 + sub * 128: c * 512 + sub * 128 + 128]
                    nc.tensor.matmul(
                        ps[:], lhsT=lhsT,
                        rhs=wv_sb[:, c * DL:(c + 1) * DL],
                        start=(c == 0), stop=(c == CC - 1))
                vbk = vb_sb[:, kcg * VW:(kcg + 1) * VW].rearrange(
                    "p (h w) -> p h w", w=128)
                nc.vector.tensor_copy(
                    vbk[:, :, 0:64],
                    ps.rearrange("p (h w) -> p h w", w=64))

        # ================= Phase B: K per pair + attention =================
        def outproj_qt(qt_i):
            for qc in range(4 * qt_i, 4 * qt_i + 4):
                for j, (mo, mn) in enumerate(nsplit(C)):
                    pf = pp.tile([128, 512], f32, tag="psp",
                                 name=f"pf{qc}_{mo}")
                    for hp2 in range(HP):
                        nc.tensor.matmul(
                            pf[:, 0:mn],
                            lhsT=ots_sb[hp2][:, qc * 128:(qc + 1) * 128],
                            rhs=wo_sb[:, hp2 * C + mo: hp2 * C + mo + mn],
                            start=(hp2 == 0), stop=(hp2 == HP - 1))
                    st = ostage.tile([128, 512], bf16, tag="fo",
                                     name=f"fo{qc}_{mo}")
                    if j == 0:
                        nc.scalar.copy(st[:, 0:mn], pf[:, 0:mn])
                    else:
                        nc.vector.tensor_copy(st[:, 0:mn], pf[:, 0:mn])
                    nc.sync.dma_start(out[qc * 128:(qc + 1) * 128, mo:mo + mn],
                                      st[:, 0:mn])

        def k_proj(hp):
            for lt in range(LT):
                xk_sb = xs.tile([128, CC * 512], bf16, tag="xk",
                                name=f"xk{hp}_{lt}")
                for c in range(CC):
                    nc.sync.dma_start(
                        xk_sb[:, c * 512:(c + 1) * 512],
                        xk[c * 128:(c + 1) * 128, lt * 512:(lt + 1) * 512])
                ps = pp.tile([128, 512], f32, tag="psp", name=f"psp_k{hp}_{lt}")
                for c in range(CC):
                    nc.tensor.matmul(
                        ps[:],
                        lhsT=wk_sb[:, c * DL + hp * 128: c * DL + hp * 128 + 128],
                        rhs=xk_sb[:, c * 512:(c + 1) * 512],
                        start=(c == 0), stop=(c == CC - 1))
                nc.scalar.copy(kt_sb[hp][:, lt * 512:(lt + 1) * 512],
                               ps[:])

        k_proj(0)
        for hp in range(HP):
            if hp == 1:
                for hp2 in range(HP):
                    nc.sync.dma_start(wo_sb[:, hp2 * C:(hp2 + 1) * C],
                                      wo[hp2 * 128:(hp2 + 1) * 128, :])

            for hi in range(2):
                h = 2 * hp + hi
                # batched scoring for the whole head
                etb = etbp.tile([128, KC * SPAN], bf16, tag="etb",
                                name=f"etb{h}")
                for kc0 in range(0, KC, 2):
                    ps = psc.tile([128, 2 * SPAN], f32, tag="sc",
                                  name=f"ps_s{h}_{kc0}")
                    for j, kc in enumerate((kc0, kc0 + 1)):
                        qs = cfg.qs_of(kc)
                        nc.tensor.matmul(
                            ps[:, j * SPAN:(j + 1) * SPAN],
                            lhsT=kt_sb[hp][hi * 64:(hi + 1) * 64,
                                           kc * 128:(kc + 1) * 128],
                            rhs=qt_sb[hp][hi * 64:(hi + 1) * 64, qs: qs + SPAN],
                            start=(j == 0), stop=(j == 1))
                    et = ets.tile([128, 2 * SPAN], bf16, tag="et",
                                  name=f"et{h}_{kc0}")
                    nc.scalar.activation(et[:], ps[:], AF.Exp, scale=0.125)
                    for j, kc in enumerate((kc0, kc0 + 1)):
                        qs = cfg.qs_of(kc)
                        seb = qs - 128 * kc + 512
                        nc.vector.tensor_mul(
                            etb[:, kc * SPAN:(kc + 1) * SPAN],
                            et[:, j * SPAN:(j + 1) * SPAN],
                            eb_sb[:, seb:seb + SPAN])
                if hi == 0 and hp + 1 < HP:
                    k_proj(hp + 1)
                # dependency-free PV stream + closes
                po = {}
                for kc in range(KC):
                    vsl = vb_sb[:, kc * VW + h * 128: kc * VW + h * 128 + 128]
                    for mm_id in mms_at[kc]:
                        _, qt_i, qoff, ncols, eoff = pv_mms[mm_id]
                        if qt_i not in po:
                            po[qt_i] = ppo.tile([128, 512], f32, tag="po",
                                                name=f"po{h}_{qt_i}")
                        nc.tensor.matmul(
                            po[qt_i][:, qoff:qoff + ncols], lhsT=vsl,
                            rhs=etb[:, kc * SPAN + eoff: kc * SPAN + eoff + ncols],
                            start=(qt_first[qt_i] == mm_id),
                            stop=(qt_last[qt_i] == mm_id))
                    for qt_i in [q for q in range(NQT) if qt_done_at[q] == kc]:
                        t = po.pop(qt_i)
                        sd = nrm.tile([64, 512], f32, tag="sd",
                                      name=f"sd{h}_{qt_i}")
                        nc.scalar.copy(sd[:], t[64:128, :])
                        rbb = nrm.tile([64, 512], f32, tag="rbb",
                                       name=f"rbb{h}_{qt_i}")
                        nc.vector.reciprocal_approx_fast(rbb[:], sd[:])
                        sl = (slice(hi * 64, (hi + 1) * 64),
                              slice(qt_i * 512, (qt_i + 1) * 512))
                        nc.vector.tensor_mul(ots_sb[hp][sl], t[0:64, :],
                                             rbb[:])
                        if h == NHL - 1:
                            outproj_qt(qt_i)

    nc.compile()
    return nc


def host_inputs(inputs, cfg=FULL):
    """Build the 8 per-core input maps + the host-side combine constant."""
    L, C, DL, NHL = cfg.L, cfg.C, cfg.DL, cfg.NHL
    q = np.asarray(inputs["queries"], np.float32)
    k = np.asarray(inputs["keys"], np.float32)
    v = np.asarray(inputs["values"], np.float32)
    Wq = np.asarray(inputs["Wq"], np.float32)
    Wk = np.asarray(inputs["Wk"], np.float32)
    Wv = np.asarray(inputs["Wv"], np.float32)
    Wo = np.asarray(inputs["Wo"], np.float32)
    bq = np.asarray(inputs["bq"], np.float32)
    bv = np.asarray(inputs["bv"], np.float32)
    bo = np.asarray(inputs["bo"], np.float32)
    B = q.shape[0]

    bo_eff = (bo.astype(np.float64) + Wo.astype(np.float64) @ bv.astype(np.float64)
              ).astype(np.float32)

    p = np.arange(128, dtype=np.float64)[:, None]
    c = np.arange(cfg.EBW, dtype=np.float64)[None, :]
    eb = np.exp(-0.1 * np.abs(p - c + 512)).astype(BF16)

    xT = {}
    for b in range(B):
        xT[b] = (np.ascontiguousarray(q[b].T).astype(BF16),
                 np.ascontiguousarray(k[b].T).astype(BF16),
                 np.ascontiguousarray(v[b].T).astype(BF16))

    in_maps = []
    for core in range(N_CORES):
        b, hg = core // 2, core % 2
        sl = slice(hg * DL, (hg + 1) * DL)
        bq_l = bq[sl].reshape(cfg.HP, 128).T  # [128, HP]
        in_maps.append({
            "xq": xT[b][0], "xk": xT[b][1], "xv": xT[b][2],
            "wq": np.ascontiguousarray(Wq.T[:, sl]).astype(BF16),
            "wk": np.ascontiguousarray(Wk.T[:, sl]).astype(BF16),
            "wv": np.ascontiguousarray(Wv.T[:, sl]).astype(BF16),
            "wo": np.ascontiguousarray(Wo.T[sl, :]).astype(BF16),
            "bq": np.ascontiguousarray(bq_l),
            "eb": eb,
        })
    return in_maps, bo_eff


_CACHED = {}


def _wait_devices_healthy(timeout_s=420):
    import time
    import jax
    import jax.numpy as jnp
    t0 = time.time()
    last = None
    while time.time() - t0 < timeout_s:
        try:
            for d in jax.devices():
                x = jax.device_put(np.ones((8, 8), np.float32), d)
                jnp.sum(x).block_until_ready()
            return
        except Exception as e:  # wedged worker recycles within a few minutes
            last = e
            time.sleep(15)
    raise RuntimeError(f"NeuronCores unhealthy after {timeout_s}s: {last}")


def kernel(**inputs):
    from concourse.bass_utils import run_bass_kernel_spmd

    cfg = FULL
    if "nc" not in _CACHED:
        _CACHED["nc"] = build_program(cfg)
    nc = _CACHED["nc"]

    in_maps, bo_eff = host_inputs(inputs, cfg)
    _wait_devices_healthy()
    try:
        res = run_bass_kernel_spmd(nc, in_maps, core_ids=list(range(N_CORES)))
    except Exception:
        _wait_devices_healthy()
        res = run_bass_kernel_spmd(nc, in_maps, core_ids=list(range(N_CORES)))
    B = np.asarray(inputs["queries"]).shape[0]
    out = np.zeros((B, cfg.L, cfg.C), np.float32)
    for b in range(B):
        out[b] = (res.results[2 * b]["out"].astype(np.float32)
                  + res.results[2 * b + 1]["out"].astype(np.float32)
                  + bo_eff[None, :])
    return out


# revision 3
# speedup vs baseline: 1.0387x; 1.0387x over previous
"""Trainium2 Bass kernel for nn_AutoCorrelation (multi-head attention with a
-0.1|i-j| distance bias), SPMD across 8 NeuronCores; core = (batch, head-half).

Design (evolved from the 294us baseline to ~224us):
 - W=64 banded attention (64-aligned 256-wide spans per 128-key chunk);
   banding error ~4e-4 against the softmax tail, budget 2e-2.
 - Per-head batched scoring: all 16 score matmuls -> paired EXP -> eb multiply
   land in a resident per-head etb strip, so the PV matmuls are a
   dependency-free stream and the next head's scoring fills PE gaps (keeps the
   HAM clock-gate warm).
 - Score psums paired: two k-chunks share one [128,512] PSUM bank and one EXP.
 - V projections and per-pair K projections (xk re-read per pair, prefetched
   one pair ahead) run inside the attention phase as independent PE filler;
   only Q runs as a prologue. Sub-tile deps let the Tile scheduler overlap.
 - Softmax denominators via a 64-wide ones block in the PV lhsT
   ([V(64) | ones(64)]): the sums appear replicated in psum rows 64-127, so
   the reciprocal/multiply are base-aligned (ACT cross-base copy 64->0, DVE
   reciprocal, DVE multiply straight off the PV psum).
 - Output projection per q-tile fires as the last head closes it; outputs are
   staged bf16 (halves store DMA), host sums the two half-core partials in
   f32 and adds bo_eff.
 - Math: bk drops out (softmax row-shift invariance); bv folded into host-side
   bo_eff = bo + Wo @ bv; exp(-0.1|i-j|) applied via a Toeplitz master array.
"""

import math
from contextlib import ExitStack

import numpy as np
import ml_dtypes

BF16 = ml_dtypes.bfloat16

N_CORES = 8


class Cfg:
    def __init__(self, L=2048, C=1024, NHL=8, DK=64, W=64):
        self.L, self.C, self.NHL, self.DK, self.W = L, C, NHL, DK, W
        self.DL = NHL * DK               # local head dims
        self.SPAN = 128 + 2 * W          # k-chunk q-span (64-aligned)
        self.KC = L // 128               # k chunks
        self.NB64 = L // 64              # 64-wide q blocks
        self.NQT = L // 512              # q tiles (512)
        self.CC = C // 128               # contraction chunks
        self.LT = L // 512               # l tiles
        self.HP = NHL // 2               # head pairs
        self.VW = NHL * 128              # V block width (V | 64x ones)
        self.EBW = self.SPAN + 512       # EB master width
        assert self.SPAN % 64 == 0 and self.SPAN <= L

    def qs_of(self, kc):
        return min(max(128 * kc - self.W, 0), self.L - self.SPAN)

    def covers64(self, qb):
        return [kc for kc in range(self.KC)
                if self.qs_of(kc) <= 64 * qb
                and self.qs_of(kc) + self.SPAN >= 64 * (qb + 1)]


FULL = Cfg(W=64)


def plan_pv(cfg):
    """64-granular PV matmul descriptors merged into per-(kc, qt) runs."""
    first_touch = {qb: min(cfg.covers64(qb)) for qb in range(cfg.NB64)}
    pv_mms = []          # (kc, qt_i, qoff, ncols, etb_off)
    qt_order = {qt: [] for qt in range(cfg.NQT)}
    for kc in range(cfg.KC):
        qs = cfg.qs_of(kc)
        qbs = [qs // 64 + j for j in range(cfg.SPAN // 64)]
        run = [qbs[0]]
        for qb in qbs[1:] + [None]:
            if (qb is not None and qb // 8 == run[0] // 8
                    and (first_touch[qb] == kc) == (first_touch[run[0]] == kc)):
                run.append(qb)
            else:
                qt_i = run[0] // 8
                qt_order[qt_i].append(len(pv_mms))
                pv_mms.append((kc, qt_i, (run[0] % 8) * 64, len(run) * 64,
                               (run[0] - qs // 64) * 64))
                run = [qb] if qb is not None else []
    qt_first = {qt: ids[0] for qt, ids in qt_order.items()}
    qt_last = {qt: ids[-1] for qt, ids in qt_order.items()}
    qt_done_at = {qt: pv_mms[ids[-1]][0] for qt, ids in qt_order.items()}
    return pv_mms, qt_first, qt_last, qt_done_at


def build_program(cfg=FULL, debug=False):
    import concourse.bass as bass
    import concourse.tile as tile
    from concourse import bacc, mybir

    f32 = mybir.dt.float32
    bf16 = mybir.dt.bfloat16
    AF = mybir.ActivationFunctionType

    L, C, NHL, DL = cfg.L, cfg.C, cfg.NHL, cfg.DL
    SPAN, KC, NQT, CC, LT, HP, VW = (cfg.SPAN, cfg.KC, cfg.NQT, cfg.CC,
                                     cfg.LT, cfg.HP, cfg.VW)

    nc = bacc.Bacc("TRN2", target_bir_lowering=False, debug=debug,
                   num_devices=N_CORES)

    xq = nc.dram_tensor("xq", [C, L], bf16, kind="ExternalInput").ap()
    xk = nc.dram_tensor("xk", [C, L], bf16, kind="ExternalInput").ap()
    xv = nc.dram_tensor("xv", [C, L], bf16, kind="ExternalInput").ap()
    wq = nc.dram_tensor("wq", [C, DL], bf16, kind="ExternalInput").ap()
    wk = nc.dram_tensor("wk", [C, DL], bf16, kind="ExternalInput").ap()
    wv = nc.dram_tensor("wv", [C, DL], bf16, kind="ExternalInput").ap()
    wo = nc.dram_tensor("wo", [DL, C], bf16, kind="ExternalInput").ap()
    bqd = nc.dram_tensor("bq", [128, HP], f32, kind="ExternalInput").ap()
    ebd = nc.dram_tensor("eb", [128, cfg.EBW], bf16, kind="ExternalInput").ap()
    out = nc.dram_tensor("out", [L, C], bf16, kind="ExternalOutput").ap()

    pv_mms, qt_first, qt_last, qt_done_at = plan_pv(cfg)
    mms_at = {kc: [i for i, m in enumerate(pv_mms) if m[0] == kc]
              for kc in range(KC)}

    def nsplit(total, cap=512):
        o, r = [], 0
        while r < total:
            n = min(cap, total - r)
            o.append((r, n))
            r += n
        return o

    with tile.TileContext(nc) as tc, ExitStack() as ctx:
        const = ctx.enter_context(tc.tile_pool(name="const", bufs=1))
        big = ctx.enter_context(tc.tile_pool(name="big", bufs=1))
        xs = ctx.enter_context(tc.tile_pool(name="xs", bufs=3))
        ets = ctx.enter_context(tc.tile_pool(name="ets", bufs=4))
        etbp = ctx.enter_context(tc.tile_pool(name="etbp", bufs=2))
        nrm = ctx.enter_context(tc.tile_pool(name="nrm", bufs=2))
        ostage = ctx.enter_context(tc.tile_pool(name="ostage", bufs=3))
        pp = ctx.enter_context(tc.tile_pool(name="pp", bufs=3, space="PSUM"))
        psc = ctx.enter_context(tc.tile_pool(name="psc", bufs=2, space="PSUM"))
        ppo = ctx.enter_context(tc.tile_pool(name="ppo", bufs=3, space="PSUM"))

        wq_sb = const.tile([128, CC * DL], bf16)
        wk_sb = const.tile([128, CC * DL], bf16)
        wv_sb = const.tile([128, CC * DL], bf16)
        wo_sb = const.tile([128, HP * C], bf16)
        eb_sb = const.tile([128, cfg.EBW], bf16)
        bq_sb = const.tile([128, HP], f32)
        qt_sb = [big.tile([128, L], bf16, name=f"qt{hp}") for hp in range(HP)]
        kt_sb = [big.tile([128, L], bf16, name=f"kt{hp}") for hp in range(HP)]
        vb_sb = big.tile([128, KC * VW], bf16)
        ots_sb = [big.tile([128, L], bf16, name=f"ots{hp}") for hp in range(HP)]

        nc.sync.dma_start(bq_sb[:], bqd[:])
        vb4 = vb_sb.rearrange("p (k h w) -> p k h w", h=NHL, w=128)
        nc.vector.memset(vb4[:, :, :, 64:128], 1.0)

        # ================= Phase A: Q projections =================
        for lt in range(LT):
            xq_sb = xs.tile([128, CC * 512], bf16, tag="xq", bufs=2, name=f"xq{lt}")
            for c in range(CC):
                if lt == 0:
                    nc.sync.dma_start(wq_sb[:, c * DL:(c + 1) * DL],
                                      wq[c * 128:(c + 1) * 128, :])
                nc.sync.dma_start(
                    xq_sb[:, c * 512:(c + 1) * 512],
                    xq[c * 128:(c + 1) * 128, lt * 512:(lt + 1) * 512])
            for hp in range(HP):
                ps = pp.tile([128, 512], f32, tag="psp", name=f"psp_q{lt}_{hp}")
                for c in range(CC):
                    nc.tensor.matmul(
                        ps[:],
                        lhsT=wq_sb[:, c * DL + hp * 128: c * DL + hp * 128 + 128],
                        rhs=xq_sb[:, c * 512:(c + 1) * 512],
                        start=(c == 0), stop=(c == CC - 1))
                nc.scalar.activation(qt_sb[hp][:, lt * 512:(lt + 1) * 512],
                                     ps[:], AF.Identity,
                                     bias=bq_sb[:, hp:hp + 1], scale=1.0)
        nc.sync.dma_start(eb_sb[:], ebd[:])
        for c in range(CC):
            nc.sync.dma_start(wv_sb[:, c * DL:(c + 1) * DL],
                              wv[c * 128:(c + 1) * 128, :])
        for c in range(CC):
            nc.sync.dma_start(wk_sb[:, c * DL:(c + 1) * DL],
                              wk[c * 128:(c + 1) * 128, :])

        # ================= Phase B prologue: V projections =================
        for lt in range(LT):
            xv_sb = xs.tile([128, CC * 512], bf16, tag="xv", bufs=2, name=f"xv{lt}")
            for c in range(CC):
                nc.sync.dma_start(
                    xv_sb[:, c * 512:(c + 1) * 512],
                    xv[c * 128:(c + 1) * 128, lt * 512:(lt + 1) * 512])
            for sub in range(4):
                kcg = lt * 4 + sub
                ps = pp.tile([128, DL], f32, tag="psp", name=f"psp_v{kcg}")
                for c in range(CC):
                    lhsT = xv_sb[:, c * 512 + sub * 128: c * 512 + sub * 128 + 128]
                    nc.tensor.matmul(
                        ps[:], lhsT=lhsT,
                        rhs=wv_sb[:, c * DL:(c + 1) * DL],
                        start=(c == 0), stop=(c == CC - 1))
                vbk = vb_sb[:, kcg * VW:(kcg + 1) * VW].rearrange(
                    "p (h w) -> p h w", w=128)
                nc.vector.tensor_copy(
                    vbk[:, :, 0:64],
                    ps.rearrange("p (h w) -> p h w", w=64))

        # ================= Phase B: K per pair + attention =================
        def outproj_qt(qt_i):
            for qc in range(4 * qt_i, 4 * qt_i + 4):
                for j, (mo, mn) in enumerate(nsplit(C)):
                    pf = pp.tile([128, 512], f32, tag="psp",
                                 name=f"pf{qc}_{mo}")
                    for hp2 in range(HP):
                        nc.tensor.matmul(
                            pf[:, 0:mn],
                            lhsT=ots_sb[hp2][:, qc * 128:(qc + 1) * 128],
                            rhs=wo_sb[:, hp2 * C + mo: hp2 * C + mo + mn],
                            start=(hp2 == 0), stop=(hp2 == HP - 1))
                    st = ostage.tile([128, 512], bf16, tag="fo",
                                     name=f"fo{qc}_{mo}")
                    if j == 0:
                        nc.scalar.copy(st[:, 0:mn], pf[:, 0:mn])
                    else:
                        nc.vector.tensor_copy(st[:, 0:mn], pf[:, 0:mn])
                    nc.sync.dma_start(out[qc * 128:(qc + 1) * 128, mo:mo + mn],
                                      st[:, 0:mn])

        def k_proj(hp):
            for lt in range(LT):
                xk_sb = xs.tile([128, CC * 512], bf16, tag="xk",
                                name=f"xk{hp}_{lt}")
                for c in range(CC):
                    nc.sync.dma_start(
                        xk_sb[:, c * 512:(c + 1) * 512],
                        xk[c * 128:(c + 1) * 128, lt * 512:(lt + 1) * 512])
                ps = pp.tile([128, 512], f32, tag="psp", name=f"psp_k{hp}_{lt}")
                for c in range(CC):
                    nc.tensor.matmul(
                        ps[:],
                        lhsT=wk_sb[:, c * DL + hp * 128: c * DL + hp * 128 + 128],
                        rhs=xk_sb[:, c * 512:(c + 1) * 512],
                        start=(c == 0), stop=(c == CC - 1))
                nc.scalar.copy(kt_sb[hp][:, lt * 512:(lt + 1) * 512],
                               ps[:])

        k_proj(0)
        for hp in range(HP):
            if hp == 1:
                for hp2 in range(HP):
                    nc.sync.dma_start(wo_sb[:, hp2 * C:(hp2 + 1) * C],
                                      wo[hp2 * 128:(hp2 + 1) * 128, :])

            for hi in range(2):
                h = 2 * hp + hi
                # batched scoring for the whole head
                etb = etbp.tile([128, KC * SPAN], bf16, tag="etb",
                                name=f"etb{h}")
                for kc0 in range(0, KC, 2):
                    ps = psc.tile([128, 2 * SPAN], f32, tag="sc",
                                  name=f"ps_s{h}_{kc0}")
                    for j, kc in enumerate((kc0, kc0 + 1)):
                        qs = cfg.qs_of(kc)
                        nc.tensor.matmul(
                            ps[:, j * SPAN:(j + 1) * SPAN],
                            lhsT=kt_sb[hp][hi * 64:(hi + 1) * 64,
                                           kc * 128:(kc + 1) * 128],
                            rhs=qt_sb[hp][hi * 64:(hi + 1) * 64, qs: qs + SPAN],
                            start=(j == 0), stop=(j == 1))
                    et = ets.tile([128, 2 * SPAN], bf16, tag="et",
                                  name=f"et{h}_{kc0}")
                    nc.scalar.activation(et[:], ps[:], AF.Exp, scale=0.125)
                    for j, kc in enumerate((kc0, kc0 + 1)):
                        qs = cfg.qs_of(kc)
                        seb = qs - 128 * kc + 512
                        nc.vector.tensor_mul(
                            etb[:, kc * SPAN:(kc + 1) * SPAN],
                            et[:, j * SPAN:(j + 1) * SPAN],
                            eb_sb[:, seb:seb + SPAN])
                if hi == 0 and hp + 1 < HP:
                    k_proj(hp + 1)
                # dependency-free PV stream + closes
                po = {}
                for kc in range(KC):
                    vsl = vb_sb[:, kc * VW + h * 128: kc * VW + h * 128 + 128]
                    for mm_id in mms_at[kc]:
                        _, qt_i, qoff, ncols, eoff = pv_mms[mm_id]
                        if qt_i not in po:
                            po[qt_i] = ppo.tile([128, 512], f32, tag="po",
                                                name=f"po{h}_{qt_i}")
                        nc.tensor.matmul(
                            po[qt_i][:, qoff:qoff + ncols], lhsT=vsl,
                            rhs=etb[:, kc * SPAN + eoff: kc * SPAN + eoff + ncols],
                            start=(qt_first[qt_i] == mm_id),
                            stop=(qt_last[qt_i] == mm_id))
                    for qt_i in [q for q in range(NQT) if qt_done_at[q] == kc]:
                        t = po.pop(qt_i)
                        sd = nrm.tile([64, 512], f32, tag="sd",
                                      name=f"sd{h}_{qt_i}")
                        nc.scalar.copy(sd[:], t[64:128, :])
                        rbb = nrm.tile([64, 512], f32, tag="rbb",
                                       name=f"rbb{h}_{qt_i}")
                        nc.vector.reciprocal_approx_fast(rbb[:], sd[:])
                        sl = (slice(hi * 64, (hi + 1) * 64),
                              slice(qt_i * 512, (qt_i + 1) * 512))
                        nc.vector.tensor_mul(ots_sb[hp][sl], t[0:64, :],
                                             rbb[:])
                        if h == NHL - 1:
                            outproj_qt(qt_i)

    nc.compile()
    return nc


def host_inputs(inputs, cfg=FULL):
    """Build the 8 per-core input maps + the host-side combine constant."""
    L, C, DL, NHL = cfg.L, cfg.C, cfg.DL, cfg.NHL
    q = np.asarray(inputs["queries"], np.float32)
    k = np.asarray(inputs["keys"], np.float32)
    v = np.asarray(inputs["values"], np.float32)
    Wq = np.asarray(inputs["Wq"], np.float32)
    Wk = np.asarray(inputs["Wk"], np.float32)
    Wv = np.asarray(inputs["Wv"], np.float32)
    Wo = np.asarray(inputs["Wo"], np.float32)
    bq = np.asarray(inputs["bq"], np.float32)
    bv = np.asarray(inputs["bv"], np.float32)
    bo = np.asarray(inputs["bo"], np.float32)
    B = q.shape[0]

    bo_eff = (bo.astype(np.float64) + Wo.astype(np.float64) @ bv.astype(np.float64)
              ).astype(np.float32)

    p = np.arange(128, dtype=np.float64)[:, None]
    c = np.arange(cfg.EBW, dtype=np.float64)[None, :]
    eb = np.exp(-0.1 * np.abs(p - c + 512)).astype(BF16)

    xT = {}
    for b in range(B):
        xT[b] = (np.ascontiguousarray(q[b].T).astype(BF16),
                 np.ascontiguousarray(k[b].T).astype(BF16),
                 np.ascontiguousarray(v[b].T).astype(BF16))

    in_maps = []
    for core in range(N_CORES):
        b, hg = core // 2, core % 2
        sl = slice(hg * DL, (hg + 1) * DL)
        bq_l = bq[sl].reshape(cfg.HP, 128).T  # [128, HP]
        in_maps.append({
            "xq": xT[b][0], "xk": xT[b][1], "xv": xT[b][2],
            "wq": np.ascontiguousarray(Wq.T[:, sl]).astype(BF16),
            "wk": np.ascontiguousarray(Wk.T[:, sl]).astype(BF16),
            "wv": np.ascontiguousarray(Wv.T[:, sl]).astype(BF16),
            "wo": np.ascontiguousarray(Wo.T[sl, :]).astype(BF16),
            "bq": np.ascontiguousarray(bq_l),
            "eb": eb,
        })
    return in_maps, bo_eff


_CACHED = {}


def _wait_devices_healthy(timeout_s=420):
    import time
    import jax
    import jax.numpy as jnp
    t0 = time.time()
    last = None
    while time.time() - t0 < timeout_s:
        try:
            for d in jax.devices():
                x = jax.device_put(np.ones((8, 8), np.float32), d)
                jnp.sum(x).block_until_ready()
            return
        except Exception as e:  # wedged worker recycles within a few minutes
            last = e
            time.sleep(15)
    raise RuntimeError(f"NeuronCores unhealthy after {timeout_s}s: {last}")


def kernel(**inputs):
    from concourse.bass_utils import run_bass_kernel_spmd

    cfg = FULL
    if "nc" not in _CACHED:
        _CACHED["nc"] = build_program(cfg)
    nc = _CACHED["nc"]

    in_maps, bo_eff = host_inputs(inputs, cfg)
    _wait_devices_healthy()
    try:
        res = run_bass_kernel_spmd(nc, in_maps, core_ids=list(range(N_CORES)))
    except Exception:
        _wait_devices_healthy()
        res = run_bass_kernel_spmd(nc, in_maps, core_ids=list(range(N_CORES)))
    B = np.asarray(inputs["queries"]).shape[0]
    out = np.zeros((B, cfg.L, cfg.C), np.float32)
    for b in range(B):
        out[b] = (res.results[2 * b]["out"].astype(np.float32)
                  + res.results[2 * b + 1]["out"].astype(np.float32)
                  + bo_eff[None, :])
    return out
